# revision 7
# baseline (speedup 1.0000x reference)
"""BiLinearAttention TRN2 Bass kernel.

Math (per batch element n, data-parallel over 8 NeuronCores):
    q_proj = query @ W.T + b          # [L, D]
    score  = q_proj @ key.T           # [L, S]
    P      = softmax(score, axis=-1)
    out    = P @ value                # [L, D]

Shapes: query/key/value [2048, 1024] f32 per core, W [1024, 1024], b [1024].

On-chip plan (per core):
  - Transpose W -> WT [q, k] and query -> queryT [q, l] via PE transposes
    (fp32 has no DMA-transpose path on TRN2).
  - q_projT [k, l] = matmul accumulation over q chunks; bias added
    on the PSUM->SBUF copy (per-partition bias on ACT).
  - Transpose key -> keyT [k, s]; both q_projT and keyT are split into
    bf16 hi/lo pairs: the score matmul runs as 3 bf16 passes
    (hi*lo + lo*hi + hi*hi accumulated in fp32 PSUM), which measures
    ~8.5e-5 rms error on a K=1024 dot -- fp32-class logits at 3/4 the
    fp32 matmul cost (fp32 matmul = 4 cycles/row on the PE).
  - Softmax over s in [l, s] layout: free-dim reduce_max on DVE,
    exp(x - max) on ACT reading PSUM directly with accum_out producing the
    denominator; P emitted as bf16.
  - P tiles PE-transposed (bf16, cheap) and P.T @ value computed in bf16
    (P is near-one-hot -- logit precision is what matters; bf16 here only
    contributes ~2e-3 relative noise against unit-scale outputs).
  - out = (P @ V) * (1/sum) via per-partition tensor_scalar on DVE.
"""

import numpy as np
from contextlib import ExitStack

import concourse.bass as bass
import concourse.tile as tile
from concourse import mybir, bacc, bass_utils
from concourse.masks import make_identity

F32 = mybir.dt.float32
BF16 = mybir.dt.bfloat16
F16 = mybir.dt.float16
AF = mybir.ActivationFunctionType
AX = mybir.AxisListType

N, L, S, D = 8, 2048, 2048, 1024
N_CORES = 8
LT = L // 128       # 16 l tiles
ST = S // 128       # 16 s tiles
KC = D // 128       # 8 contraction chunks (both q and k dims)
SB = S // 512       # 4 score blocks per l tile
LB = L // 512       # 4 l blocks in projection
DB = D // 512       # 2 d blocks in PV

# score matmul mode: "split" (3-pass bf16 hi/lo, fp32-class) or "f32"
SCORE_MODE = "split"


def _alt_copy(nc, i, out, in_):
    """Alternate PSUM->SBUF copies between DVE and ACT to balance load."""
    if i % 2 == 0:
        nc.vector.tensor_copy(out, in_)
    else:
        nc.scalar.copy(out, in_)


def _emit(ctx: ExitStack, tc: tile.TileContext,
          query, key, value, W, b, out):
    nc = tc.nc

    base = ctx.enter_context(tc.tile_pool(name="base", bufs=1))
    ident32 = base.tile([128, 128], F32)
    make_identity(nc, ident32)
    ident16 = base.tile([128, 128], F16)
    make_identity(nc, ident16)
    b_sb = base.tile([128, KC], F32)
    nc.sync.dma_start(b_sb, b.rearrange("(t p) -> p t", p=128))

    # persistent attention operands
    p_qp = ctx.enter_context(tc.tile_pool(name="qp", bufs=1))
    if SCORE_MODE == "split":
        qpT_hi = p_qp.tile([128, KC, L], BF16)
        qpT_lo = p_qp.tile([128, KC, L], BF16)
    else:
        qpT_f32 = p_qp.tile([128, KC, L], F32)

    # ---------------- phases 0-3 (own PSUM scope) ----------------
    with tc.tile_pool(name="ps_tr", bufs=4, space="PSUM") as ps_tr, \
         tc.tile_pool(name="ps_mm", bufs=4, space="PSUM") as ps_mm:

        with tc.tile_pool(name="wt", bufs=1) as p_wt, \
             tc.tile_pool(name="stream", bufs=3) as p_stream, \
             tc.tile_pool(name="qtb", bufs=2) as p_qtb, \
             tc.tile_pool(name="tmp", bufs=3) as p_tmp:
            # WT[q, k] = W[k, q].T built from streamed W row tiles
            WT = p_wt.tile([128, KC, D], F32)      # [q_in_chunk, q_chunk, k]
            ci = 0
            for kt in range(KC):
                wnat = p_stream.tile([128, D], F32, tag="wnat")
                nc.sync.dma_start(wnat, W[kt * 128:(kt + 1) * 128, :])
                for qc in range(KC):
                    tp = ps_tr.tile([128, 128], F32, tag="tr")
                    nc.tensor.transpose(tp, wnat[:, qc * 128:(qc + 1) * 128],
                                        ident32)
                    _alt_copy(nc, ci, WT[:, qc, kt * 128:(kt + 1) * 128], tp)
                    ci += 1

            # per l-block: transpose query block, then project it
            for lb in range(LB):
                qT = p_qtb.tile([128, KC, 512], F32, tag="qT")
                for i in range(4):
                    lt = lb * 4 + i
                    qnat = p_stream.tile([128, D], F32, tag="qnat")
                    nc.sync.dma_start(qnat, query[lt * 128:(lt + 1) * 128, :])
                    for qc in range(KC):
                        tp = ps_tr.tile([128, 128], F32, tag="tr")
                        nc.tensor.transpose(
                            tp, qnat[:, qc * 128:(qc + 1) * 128], ident32)
                        _alt_copy(nc, ci, qT[:, qc, i * 128:(i + 1) * 128], tp)
                        ci += 1

                # q_projT[k, l] = sum_q WT[q, k] * queryT[q, l]  (fp32)
                for kt in range(KC):
                    mm = ps_mm.tile([128, 512], F32, tag="mm")
                    for qc in range(KC):
                        nc.tensor.matmul(
                            mm,
                            WT[:, qc, kt * 128:(kt + 1) * 128],
                            qT[:, qc, :],
                            start=(qc == 0), stop=(qc == KC - 1),
                        )
                    bias = b_sb[:, kt:kt + 1]
                    sl = slice(lb * 512, (lb + 1) * 512)
                    if SCORE_MODE == "split":
                        qp32 = p_tmp.tile([128, 512], F32, tag="qp32")
                        nc.scalar.activation(qp32, mm, AF.Identity,
                                             bias=bias, scale=1.0)
                        nc.vector.tensor_copy(qpT_hi[:, kt, sl], qp32)
                        hi32 = p_tmp.tile([128, 512], F32, tag="hi32")
                        nc.scalar.copy(hi32, qpT_hi[:, kt, sl])
                        nc.vector.tensor_sub(qpT_lo[:, kt, sl], qp32, hi32)
                    else:
                        nc.scalar.activation(qpT_f32[:, kt, sl], mm, AF.Identity,
                                             bias=bias, scale=1.0)

        # ---------------- phase 3: keyT (hi/lo) + value (bf16) ----------
        p_kv = ctx.enter_context(tc.tile_pool(name="kv", bufs=1))
        if SCORE_MODE == "split":
            keyT_hi = p_kv.tile([128, KC, S], BF16)
            keyT_lo = p_kv.tile([128, KC, S], BF16)
        else:
            keyT_f32 = p_kv.tile([128, KC, S], F32)
        v_sb = p_kv.tile([128, ST, D], F16)
        # SWDGE cast f32 -> bf16 on the way in; 4 chunks for overlap
        for vc in range(4):
            nc.gpsimd.dma_start(
                v_sb[:, vc * 4:(vc + 1) * 4, :],
                value.rearrange("(t p) d -> p t d", p=128)[:, vc * 4:(vc + 1) * 4, :])

        with tc.tile_pool(name="stream2", bufs=3) as p_stream2, \
             tc.tile_pool(name="tmp2", bufs=4) as p_tmp2:
            ci = 0
            for st in range(ST):
                knat = p_stream2.tile([128, D], F32, tag="knat")
                nc.sync.dma_start(knat, key[st * 128:(st + 1) * 128, :])
                for kc in range(KC):
                    tp = ps_tr.tile([128, 128], F32, tag="tr")
                    nc.tensor.transpose(tp, knat[:, kc * 128:(kc + 1) * 128],
                                        ident32)
                    sl = slice(st * 128, (st + 1) * 128)
                    if SCORE_MODE == "split":
                        kt32 = p_tmp2.tile([128, 128], F32, tag="kt32")
                        _alt_copy(nc, ci, kt32, tp)
                        nc.vector.tensor_copy(keyT_hi[:, kc, sl], kt32)
                        khi32 = p_tmp2.tile([128, 128], F32, tag="khi32")
                        nc.scalar.copy(khi32, keyT_hi[:, kc, sl])
                        nc.vector.tensor_sub(keyT_lo[:, kc, sl], kt32, khi32)
                    else:
                        _alt_copy(nc, ci, keyT_f32[:, kc, sl], tp)
                    ci += 1

    # ---------------- phase 4: attention over l tiles ----------------
    ps_score = ctx.enter_context(tc.tile_pool(name="ps_s", bufs=4, space="PSUM"))
    ps_out = ctx.enter_context(tc.tile_pool(name="ps_o", bufs=2, space="PSUM"))
    ps_pt = ctx.enter_context(tc.tile_pool(name="ps_pt", bufs=2, space="PSUM"))
    p_p = ctx.enter_context(tc.tile_pool(name="p_p", bufs=2))
    p_pt = ctx.enter_context(tc.tile_pool(name="p_pt", bufs=6))
    p_stat = ctx.enter_context(tc.tile_pool(name="p_stat", bufs=3))
    p_out = ctx.enter_context(tc.tile_pool(name="p_out", bufs=2))

    def emit_score_softmax(lt):
        """Score matmuls + softmax for l tile lt. Returns P tile + 1/sum."""
        score_ps = []
        mx4 = p_stat.tile([128, SB], F32, tag="mx4")
        for sb in range(SB):
            mm = ps_score.tile([128, 512], F32, tag="sc")
            ssl = slice(sb * 512, (sb + 1) * 512)
            lsl = slice(lt * 128, (lt + 1) * 128)
            if SCORE_MODE == "split":
                nmm = 3 * KC
                i = 0
                for kc in range(KC):
                    for a, bb in ((qpT_hi, keyT_lo), (qpT_lo, keyT_hi),
                                  (qpT_hi, keyT_hi)):
                        nc.tensor.matmul(mm, a[:, kc, lsl], bb[:, kc, ssl],
                                         start=(i == 0), stop=(i == nmm - 1))
                        i += 1
            else:
                for kc in range(KC):
                    nc.tensor.matmul(mm, qpT_f32[:, kc, lsl],
                                     keyT_f32[:, kc, ssl],
                                     start=(kc == 0), stop=(kc == KC - 1))
            nc.vector.reduce_max(mx4[:, sb:sb + 1], mm, axis=AX.X)
            score_ps.append(mm)

        nm = p_stat.tile([128, 1], F32, tag="nm")
        # nm = -(max) + ln(2^10): P is emitted scaled by 1024 to keep the
        # fp16 tail out of denormals; the softmax normalizer absorbs it.
        nc.vector.reduce_max(nm, mx4, axis=AX.X, negate=True)
        nc.vector.tensor_scalar_add(nm, nm, float(np.log(1024.0)))
        p_sb = p_p.tile([128, S], F16, tag="p")
        ssum4 = p_stat.tile([128, SB], F32, tag="ssum4")
        for sb in range(SB):
            nc.scalar.activation(p_sb[:, sb * 512:(sb + 1) * 512], score_ps[sb],
                                 AF.Exp, bias=nm, scale=1.0,
                                 accum_out=ssum4[:, sb:sb + 1])
        ssum = p_stat.tile([128, 1], F32, tag="ssum")
        nc.vector.reduce_sum(ssum, ssum4, axis=AX.X)
        rinv = p_stat.tile([128, 1], F32, tag="rinv")
        nc.vector.reciprocal(rinv, ssum)
        return p_sb, rinv

    def emit_ptpv(lt, p_sb, rinv):
        """Transpose P tiles, accumulate P.T-weighted V, scale, store."""
        out_ps = [ps_out.tile([128, 512], F32, tag="o", name=f"ops{lt}_{i}")
                  for i in range(DB)]
        for sc in range(ST):
            tp = ps_pt.tile([128, 128], F16, tag="pt")
            nc.tensor.transpose(tp, p_sb[:, sc * 128:(sc + 1) * 128], ident16)
            pt_sb = p_pt.tile([128, 128], F16, tag="pts")
            _alt_copy(nc, sc, pt_sb, tp)
            for dc in range(DB):
                nc.tensor.matmul(out_ps[dc], pt_sb,
                                 v_sb[:, sc, dc * 512:(dc + 1) * 512],
                                 start=(sc == 0), stop=(sc == ST - 1))
        o_sb = p_out.tile([128, D], F32, tag="osb")
        for dc in range(DB):
            nc.vector.tensor_scalar_mul(o_sb[:, dc * 512:(dc + 1) * 512],
                                        out_ps[dc], rinv)
        nc.sync.dma_start(out[lt * 128:(lt + 1) * 128, :], o_sb)

    pending = None
    for lt in range(LT):
        cur = emit_score_softmax(lt)
        if pending is not None:
            emit_ptpv(lt - 1, *pending)
        pending = cur
    emit_ptpv(LT - 1, *pending)


_CACHE = {}


def _build():
    key_ = SCORE_MODE
    if key_ in _CACHE:
        return _CACHE[key_]
    nc = bacc.Bacc("TRN2", target_bir_lowering=False, debug=False,
                   num_devices=N_CORES)
    query = nc.dram_tensor("query", [L, D], F32, kind="ExternalInput").ap()
    key = nc.dram_tensor("key", [S, D], F32, kind="ExternalInput").ap()
    value = nc.dram_tensor("value", [S, D], F32, kind="ExternalInput").ap()
    W = nc.dram_tensor("W", [D, D], F32, kind="ExternalInput").ap()
    b = nc.dram_tensor("b", [D], F32, kind="ExternalInput").ap()
    out = nc.dram_tensor("out", [L, D], F32, kind="ExternalOutput").ap()
    with tile.TileContext(nc) as tc:
        with ExitStack() as ctx:
            _emit(ctx, tc, query, key, value, W, b, out)
    nc.compile()
    _CACHE[key_] = nc
    return nc


def kernel(key, query, value, W, b):
    key = np.ascontiguousarray(np.asarray(key), dtype=np.float32)
    query = np.ascontiguousarray(np.asarray(query), dtype=np.float32)
    value = np.ascontiguousarray(np.asarray(value), dtype=np.float32)
    W = np.ascontiguousarray(np.asarray(W), dtype=np.float32)
    b = np.ascontiguousarray(np.asarray(b), dtype=np.float32)
    nc = _build()
    in_maps = [
        {"query": query[i], "key": key[i], "value": value[i], "W": W, "b": b}
        for i in range(N_CORES)
    ]
    res = bass_utils.run_bass_kernel_spmd(nc, in_maps, core_ids=list(range(N_CORES)))
    return np.stack([res.results[i]["out"] for i in range(N_CORES)], axis=0)


# revision 28
# speedup vs baseline: 117.0671x; 117.0671x over previous
"""BiLinearAttention TRN2 Bass kernel.

Math (per batch element n, data-parallel over 8 NeuronCores):
    q_proj = query @ W.T + b          # [L, D]
    score  = q_proj @ key.T           # [L, S]
    P      = softmax(score, axis=-1)
    out    = P @ value                # [L, D]

Shapes: query/key/value [2048, 1024] f32 per core, W [1024, 1024], b [1024].

Design notes (all HW-verified on TRN2):
  - fp32 matmuls cost 4 cycles/row on the PE; 16-bit matmuls cost 1.
    Every fp32 operand is split into an fp16 hi/lo pair (hi = fp16(x),
    lo = fp16(x - hi)) and each contraction runs as 3 fp16 passes
    (hi*lo + lo*hi + hi*hi) accumulated in fp32 PSUM: measured 3.4e-7 rms
    error on a K=1024 dot at W-scale -- fp32-class accuracy at 3/4 the
    fp32 matmul cost. Logit accuracy matters here: score std is ~45 and
    top-2 gaps ~11, so softmax is a near-argmax; bf16/fp32r logits
    visibly corrupt the output.
  - No PE transposes: operands are split in natural layout (cheap
    free-dim DVE/ACT ops) and moved to contraction-major layout with the
    2-byte X-bar DMA transpose, batched as one [128, F] -> [128, F/128,
    128] descriptor set per tile row.
  - Engine-queue discipline: a sequencer blocks on its current
    instruction's semaphore waits, so dependent DMAs interleaved on one
    queue serialize the whole prep pipeline. Prep loads issue in groups
    of 4 ahead of the group's X-bars; X-bar transposes all stay on SP
    (concurrent X-bar streams from two HWDGE queues corrupt data --
    HW-verified); stores ride GPSIMD/SWDGE.
  - Softmax over s in [l, s] layout: free-dim reduce_max on DVE, exp on
    ACT reading score PSUM directly, with accum_out producing the
    denominator. P is emitted as fp16 scaled by 2^10 (folded into the
    exp bias; the normalizer absorbs it) to keep the tail of the
    near-one-hot distribution out of fp16 denormals.
  - P tiles X-bar-transposed, P.T @ value in fp16, then
    out = psum * (1/sum) via per-partition tensor_scalar on DVE.
"""

import numpy as np
from contextlib import ExitStack

import concourse.bass as bass
import concourse.tile as tile
from concourse import mybir, bacc, bass_utils

F32 = mybir.dt.float32
F16 = mybir.dt.float16
AF = mybir.ActivationFunctionType
AX = mybir.AxisListType

N, L, S, D = 8, 2048, 2048, 1024
N_CORES = 8
LT = L // 128       # 16 l tiles
ST = S // 128       # 16 s tiles
KC = D // 128       # 8 contraction chunks (both q and k dims)
SB = S // 512       # 4 score blocks per l tile
LB = L // 512       # 4 l blocks in projection
DB = D // 512       # 2 d blocks in PV

PSCALE = float(np.log(1024.0))


def _emit(ctx: ExitStack, tc: tile.TileContext,
          query, key, value, W, b, out, loop_T=0):
    nc = tc.nc
    _emit.uid = getattr(_emit, "uid", 0)

    base = ctx.enter_context(tc.tile_pool(name="base", bufs=1))
    b_sb = base.tile([128, KC], F32)
    nc.gpsimd.dma_start(b_sb, b.rearrange("(t p) -> p t", p=128))

    # q_projT fp16 pairs, [k_in_chunk, k_chunk, l_quarter] -- persistent
    p_qp = ctx.enter_context(tc.tile_pool(name="qp", bufs=1))
    qpT_hi = [p_qp.tile([128, KC, 512], F16, name=f"qpThi{i}") for i in range(LB)]
    qpT_lo = [p_qp.tile([128, KC, 512], F16, name=f"qpTlo{i}") for i in range(LB)]

    # first quarter of keyT hi/lo pair (combined layout [128, 2, KC, 512])
    p_kv1 = ctx.enter_context(tc.tile_pool(name="kv1", bufs=1))
    kT = [p_kv1.tile([128, 2, KC, 512], F16, name="kT0")]

    def split_nat(src_f32, hi_dst, lo_dst):
        """hi = fp16(x); lo = fp16(x - hi) via mixed-dtype DVE sub."""
        nc.vector.tensor_copy(hi_dst, src_f32)
        nc.vector.tensor_sub(lo_dst, src_f32, hi_dst)

    def load_split_xbar_group(p_stream, p_splt, items):
        """Batch of (src_rows, T_dst, fsl) where T_dst is a combined
        [128, 2, KC, F] hi/lo tile. Loads all issue before any xbar so
        no sequencer stalls a load behind an earlier xbar's wait; hi+lo
        transpose in ONE xbar DMA per row tile."""
        pairs = []
        for src_rows, T_dst, fsl in items:
            nat = p_stream.tile([128, D], F32, tag="nat",
                                name=f"nat{_emit.uid}")
            _emit.uid += 1
            nc.sync.dma_start(nat, src_rows)
            pairs.append(nat)
        outs = []
        for nat, (src_rows, T_dst, fsl) in zip(pairs, items):
            hl = p_splt.tile([128, 2, D], F16, tag="hl16",
                             name=f"hl16_{_emit.uid}")
            _emit.uid += 1
            split_nat(nat, hl[:, 0, :], hl[:, 1, :])
            outs.append(hl)
        for hl, (src_rows, T_dst, fsl) in zip(outs, items):
            nc.sync.dma_start(T_dst[:, :, :, fsl],
                              hl.rearrange("p a d -> p (a d)"),
                              transpose=True)

    # ------- phase A: W/query pairs + projection (keys 0-1 overlapped) ----
    with tc.tile_pool(name="wt", bufs=1) as p_wt, \
         tc.tile_pool(name="stream", bufs=6) as p_stream, \
         tc.tile_pool(name="splt", bufs=5) as p_splt, \
         tc.tile_pool(name="qps", bufs=3) as p_qps, \
         tc.tile_pool(name="qtb", bufs=2) as p_qtb, \
         tc.tile_pool(name="ps_mm", bufs=4, space="PSUM") as ps_mm:

        # per-kt combined WT tiles: first proj matmuls of k-tile kt only
        # depend on W row-tile kt's single xbar
        WT = [p_wt.tile([128, 2, KC, 128], F16, name=f"WT{kt}")
              for kt in range(KC)]
        for g in range(2):
            load_split_xbar_group(p_stream, p_splt, [
                (W[kt * 128:(kt + 1) * 128, :], WT[kt], slice(0, 128))
                for kt in range(g * 4, (g + 1) * 4)])

        for lb in range(LB):
            # query block -> combined fp16 pair in [q, l_block] layout
            qT = p_qtb.tile([128, 2, KC, 512], F16, tag="qT")
            load_split_xbar_group(p_stream, p_splt, [
                (query[(lb * 4 + i) * 128:(lb * 4 + i + 1) * 128, :],
                 qT, slice(i * 128, (i + 1) * 128))
                for i in range(4)])

            # q_projT[k, l_blk] = sum_q W[k, q] * queryT[q, l_blk]
            for kt in range(KC):
                mm = ps_mm.tile([128, 512], F32, tag="mm")
                i = 0
                for qc in range(KC):
                    for uc, vc in ((0, 1), (1, 0), (0, 0)):
                        nc.tensor.matmul(
                            mm,
                            WT[kt][:, uc, qc, :],
                            qT[:, vc, qc, :],
                            start=(i == 0), stop=(i == 3 * KC - 1),
                        )
                        i += 1
                qp32 = p_qps.tile([128, 512], F32, tag="qp32")
                nc.scalar.activation(qp32, mm, AF.Identity,
                                     bias=b_sb[:, kt:kt + 1], scale=1.0)
                split_nat(qp32, qpT_hi[lb][:, kt, :], qpT_lo[lb][:, kt, :])

        # key quarter 0: loads/splits/xbars overlap proj on other engines
        load_split_xbar_group(p_stream, p_splt, [
            (key[st * 128:(st + 1) * 128, :], kT[0],
             slice(st * 128, (st + 1) * 128))
            for st in range(4)])

    # ------- phase B: key quarters 2-3 + value fp16 -------
    p_kv2 = ctx.enter_context(tc.tile_pool(name="kv2", bufs=1))
    kT += [p_kv2.tile([128, 2, KC, 512], F16, name=f"kT{i}") for i in (1, 2, 3)]
    v_sb = [p_kv2.tile([128, 4, D], F16, name=f"vsb{i}") for i in range(4)]

    with tc.tile_pool(name="stream2", bufs=4) as p_stream2, \
         tc.tile_pool(name="splt2", bufs=4) as p_splt2:
        for q4 in range(1, 4):
            load_split_xbar_group(p_stream2, p_splt2, [
                (key[(q4 * 4 + r4) * 128:(q4 * 4 + r4 + 1) * 128, :],
                 kT[q4], slice(r4 * 128, (r4 + 1) * 128))
                for r4 in range(4)])
        for vq in range(4):
            nc.gpsimd.dma_start(
                v_sb[vq],
                value.rearrange("(t p) d -> p t d", p=128)[:, vq * 4:(vq + 1) * 4, :])

    # ------- phase C: attention over l tiles -------
    ps_score = ctx.enter_context(tc.tile_pool(name="ps_s", bufs=5, space="PSUM"))
    ps_out = ctx.enter_context(tc.tile_pool(name="ps_o", bufs=2, space="PSUM"))
    p_p = ctx.enter_context(tc.tile_pool(name="p_p", bufs=2))
    p_pt = ctx.enter_context(tc.tile_pool(name="p_pt", bufs=2))
    p_stat = ctx.enter_context(tc.tile_pool(name="p_stat", bufs=3))
    p_out = ctx.enter_context(tc.tile_pool(name="p_out", bufs=2))

    def emit_score_softmax(lt):
        """Score matmuls + softmax for l tile lt; returns (PT, 1/sum)."""
        score_ps = []
        mx4 = p_stat.tile([128, SB], F32, tag="mx4")
        lb, li = divmod(lt, 4)
        lsl = slice(li * 128, (li + 1) * 128)
        for sb in range(SB):
            mm = ps_score.tile([128, 512], F32, tag="sc")
            i = 0
            for kc in range(KC):
                for u, vc in ((qpT_hi[lb], 1), (qpT_lo[lb], 0),
                              (qpT_hi[lb], 0)):
                    nc.tensor.matmul(mm, u[:, kc, lsl], kT[sb][:, vc, kc, :],
                                     start=(i == 0), stop=(i == 3 * KC - 1))
                    i += 1
            nc.vector.reduce_max(mx4[:, sb:sb + 1], mm, axis=AX.X)
            score_ps.append(mm)

        nm = p_stat.tile([128, 1], F32, tag="nm")
        # nm = -(max) + ln(2^10): P scaled by 1024 (normalizer absorbs it)
        nc.vector.reduce_max(nm, mx4, axis=AX.X, negate=True)
        nc.vector.tensor_scalar_add(nm, nm, PSCALE)
        p_sb = p_p.tile([128, S], F16, tag="p")
        ssum4 = p_stat.tile([128, SB], F32, tag="ssum4")
        for sb in range(SB):
            nc.scalar.activation(p_sb[:, sb * 512:(sb + 1) * 512], score_ps[sb],
                                 AF.Exp, bias=nm, scale=1.0,
                                 accum_out=ssum4[:, sb:sb + 1])
        ssum = p_stat.tile([128, 1], F32, tag="ssum")
        nc.vector.reduce_sum(ssum, ssum4, axis=AX.X)
        rinv = p_stat.tile([128, 1], F32, tag="rinv")
        nc.vector.reciprocal(rinv, ssum)
        # PT[s', sc, l'] = P[l', sc*128+s'] -- one batched xbar transpose
        pt = p_pt.tile([128, ST, 128], F16, tag="pt")
        nc.sync.dma_start(pt, p_sb, transpose=True)
        return pt, rinv

    def emit_pv(lt, pt, rinv):
        """P.T-weighted V accumulation, scale, store."""
        out_ps = [ps_out.tile([128, 512], F32, tag="o", name=f"ops{lt}_{i}")
                  for i in range(DB)]
        for sc in range(ST):
            for dc in range(DB):
                nc.tensor.matmul(out_ps[dc], pt[:, sc, :],
                                 v_sb[sc // 4][:, sc % 4, dc * 512:(dc + 1) * 512],
                                 start=(sc == 0), stop=(sc == ST - 1))
        o_sb = p_out.tile([128, D], F32, tag="osb")
        for dc in range(DB):
            nc.vector.tensor_scalar_mul(o_sb[:, dc * 512:(dc + 1) * 512],
                                        out_ps[dc], rinv)
        nc.gpsimd.dma_start(out[lt * 128:(lt + 1) * 128, :], o_sb)

    def phase4():
        pending = None
        for lt in range(LT):
            cur = emit_score_softmax(lt)
            if pending is not None:
                emit_pv(lt - 1, *pending)
            pending = cur
        emit_pv(LT - 1, *pending)

    if loop_T:
        with tc.For_i(0, loop_T, 1):
            phase4()
    else:
        phase4()


_CACHE = {}


def _build(reps=1, loop_T=0, loop_all=0):
    key_ = (reps, loop_T, loop_all)
    if key_ in _CACHE:
        return _CACHE[key_]
    nc = bacc.Bacc("TRN2", target_bir_lowering=False, debug=False,
                   num_devices=N_CORES)
    query = nc.dram_tensor("query", [L, D], F32, kind="ExternalInput").ap()
    key = nc.dram_tensor("key", [S, D], F32, kind="ExternalInput").ap()
    value = nc.dram_tensor("value", [S, D], F32, kind="ExternalInput").ap()
    W = nc.dram_tensor("W", [D, D], F32, kind="ExternalInput").ap()
    b = nc.dram_tensor("b", [D], F32, kind="ExternalInput").ap()
    out = nc.dram_tensor("out", [L, D], F32, kind="ExternalOutput").ap()
    tag = None
    loop_T = loop_T or loop_all
    if reps > 1 or loop_T:
        # distinct I/O signature per variant so the neuron compile cache
        # (keyed on HLO structure, not backend_config) can't collide
        tag = nc.dram_tensor("tag", [8, reps * 100 + max(loop_T, 1)], F32,
                             kind="ExternalOutput").ap()
    with tile.TileContext(nc) as tc:
        if loop_all:
            with tc.For_i(0, loop_all, 1):
                with ExitStack() as ctx:
                    _emit(ctx, tc, query, key, value, W, b, out)
        else:
            for _ in range(reps):
                with ExitStack() as ctx:
                    _emit(ctx, tc, query, key, value, W, b, out, loop_T=loop_T)
        if tag is not None:
            with tc.tile_pool(name="tagp", bufs=1) as tp:
                t = tp.tile([8, reps * 100 + max(loop_T, 1)], F32)
                nc.vector.memset(t, 1.0)
                nc.sync.dma_start(tag, t)
    nc.compile()
    _CACHE[key_] = nc
    return nc


def kernel(key, query, value, W, b):
    key = np.ascontiguousarray(np.asarray(key), dtype=np.float32)
    query = np.ascontiguousarray(np.asarray(query), dtype=np.float32)
    value = np.ascontiguousarray(np.asarray(value), dtype=np.float32)
    W = np.ascontiguousarray(np.asarray(W), dtype=np.float32)
    b = np.ascontiguousarray(np.asarray(b), dtype=np.float32)
    nc = _build()
    in_maps = [
        {"query": query[i], "key": key[i], "value": value[i], "W": W, "b": b}
        for i in range(N_CORES)
    ]
    res = bass_utils.run_bass_kernel_spmd(nc, in_maps, core_ids=list(range(N_CORES)))
    return np.stack([res.results[i]["out"] for i in range(N_CORES)], axis=0)
